# revision 12
# baseline (speedup 1.0000x reference)
"""Trainium2 Bass kernel for nn_AttPCB (grouped 6-token attention block).

Math (per sample n):
  x   = parts_feat[n,:,:,0]                      # [C=2048, P=6]
  q/k/v = W x + b                                # [D=512, 6]
  per group g (8 groups of 64 channels):
    qk = (Qg^T Kg) / 8 ; attn = softmax(qk, -1)  # [6, 6]
    out_g = Vg @ attn^T                          # [64, 6]
  o  = wo @ out + bo                             # [2048, 6]
  ret = x + o

Distribution: pure data parallel over N=4096 samples across 8 cores
(512 samples/core).  Weights are replicated.

Key design points (v2):
  * QKV projections run as fp8e4m3 DoubleRow matmuls (0.5 cyc/row) using a
    3-term hi/lo error split: x*w ~= x_hi*w_hi + x_hi*w_lo + x_lo*w_hi,
    which costs 0.75x the bf16 rows with near-bf16 accuracy (rel err
    ~3e-3 end to end).  w is pre-scaled by 64 on the host so its hi/lo fp8
    planes stay in the normal range; the PSUM->SBUF copy rescales by 1/64.
  * x is uploaded pre-transposed (c-major, block-major) as fp8 hi/lo
    planes, eliminating all PE transposes of x and their PSUM->SBUF hops.
  * The residual stream xb = bf16(x + bo_eff) is uploaded n-major; the
    output-projection bias is folded into it (with the bias-feedback
    corrections bq_eff/bo_eff computed on the host), the k bias is
    softmax-invariant and dropped, and the v bias folds into bo_eff.
    q bias enters as a K=1 fp8 DoubleRow ones-row matmul.
  * Attention math stays bf16 on DVE (2x mode): qk products + log2
    add-trees, per-p softmax without max-subtraction, attn*v with a
    ScalarE broadcast-expand.  out^T is PE-transposed to d-major for the
    bf16 output projection; the residual add reads pso straight from PSUM
    and accumulates into the xb tiles, which are then stored as bf16.
  * I/O is bf16/fp8 end to end (~37.8 MB/core vs 75.5 fp32), with the
    final bf16->fp32 widening done on the host.
LDWEIGHTS deduplication runs as a post-pass before compile.
"""

import numpy as np
import ml_dtypes

N_FULL = 4096
C = 2048
P = 6
D = 512
G = 8
FD = 64
NCORES = 8
NB = 128          # samples per block
CP = C * P        # 12288
QF = CP // 4      # free elems per c-quarter tile (3072)
TOK = NB * P      # tokens per block (768)
NPAIR = 8         # 128-channel chunk pairs in C
WS = 64.0         # host-side scale on the QKV weight planes

_CACHE = {}


def _build(ns, reps=1):
    """Build the Bass/Tile program for one core processing ns samples."""
    from contextlib import ExitStack

    import concourse.bass as bass
    import concourse.tile as tile
    import concourse.mybir as mybir
    from concourse import bacc
    from concourse.bass_types import AP
    from concourse.masks import make_identity

    f32 = mybir.dt.float32
    bf16 = mybir.dt.bfloat16
    fp8 = mybir.dt.float8e4
    MULT = mybir.AluOpType.mult
    ADD = mybir.AluOpType.add
    AX = mybir.AxisListType.X
    COPY = mybir.ActivationFunctionType.Copy
    EXP = mybir.ActivationFunctionType.Exp
    DR = mybir.MatmulPerfMode.DoubleRow

    assert ns % NB == 0
    nblocks = ns // NB

    nc = bacc.Bacc("TRN2", target_bir_lowering=False, debug=False)

    xhi_d = nc.dram_tensor("xhi", [nblocks * C, TOK], fp8, kind="ExternalInput")
    xlo_d = nc.dram_tensor("xlo", [nblocks * C, TOK], fp8, kind="ExternalInput")
    xb_d = nc.dram_tensor("xb", [ns, CP], bf16, kind="ExternalInput")
    whi_d = nc.dram_tensor("whi", [C, 3 * D], fp8, kind="ExternalInput")
    wlo_d = nc.dram_tensor("wlo", [C, 3 * D], fp8, kind="ExternalInput")
    woT_d = nc.dram_tensor("woT", [D, C], bf16, kind="ExternalInput")
    ones8_d = nc.dram_tensor("ones8", [1, 256], fp8, kind="ExternalInput")
    bq8_d = nc.dram_tensor("bq8", [1, 1024], fp8, kind="ExternalInput")
    out_d = nc.dram_tensor("out", [ns, CP], bf16, kind="ExternalOutput")

    def ap(tile_ap, off, dims):
        """Custom access pattern into a tile: dims = [[step,count],...]."""
        return AP(tile_ap.tensor, tile_ap.offset + off, dims)

    with ExitStack() as ctx:
        tc = ctx.enter_context(tile.TileContext(nc))

        # ---- persistent weights / constants ----
        wpool = ctx.enter_context(tc.tile_pool(name="w8", bufs=16))
        wopool = ctx.enter_context(tc.tile_pool(name="woT", bufs=4))
        cpool = ctx.enter_context(tc.tile_pool(name="const", bufs=1))

        # fp8 weight pair tiles: [128, (chunk 2) x (proj 3 x 512)]
        whi_sb = [wpool.tile([128, 2 * 3 * D], fp8, name="whi", tag="whi")
                  for _ in range(NPAIR)]
        wlo_sb = [wpool.tile([128, 2 * 3 * D], fp8, name="wlo", tag="wlo")
                  for _ in range(NPAIR)]
        woT_sb = [wopool.tile([128, C], bf16, name="woTsb", tag="woT")
                  for _ in range(4)]

        def load_weights():
            # emitted after block 0's x DMAs so x streams in first
            for i in range(NPAIR):
                src = AP(whi_d, (2 * i * 128) * 3 * D,
                         [[3 * D, 128], [128 * 3 * D, 2], [1, 3 * D]])
                nc.sync.dma_start(
                    ap(whi_sb[i][:], 0, [[2 * 3 * D, 128], [3 * D, 2], [1, 3 * D]]),
                    src)
                src = AP(wlo_d, (2 * i * 128) * 3 * D,
                         [[3 * D, 128], [128 * 3 * D, 2], [1, 3 * D]])
                nc.sync.dma_start(
                    ap(wlo_sb[i][:], 0, [[2 * 3 * D, 128], [3 * D, 2], [1, 3 * D]]),
                    src)
            for dc in range(4):
                nc.sync.dma_start(woT_sb[dc][:], woT_d.ap()[dc * 128:(dc + 1) * 128, :])

        ones8_sb = cpool.tile([1, 256], fp8, tag="ones8")
        bq8_sb = cpool.tile([1, 1024], fp8, tag="bq8")
        identb = cpool.tile([128, 128], bf16, tag="identb")
        nc.sync.dma_start(ones8_sb[:], ones8_d.ap()[:, :])
        nc.sync.dma_start(bq8_sb[:], bq8_d.ap()[:, :])
        make_identity(nc, identb[:])

        # ---- per-block pools ----
        x8_pool = ctx.enter_context(tc.tile_pool(name="x8", bufs=19))
        xb_pool = ctx.enter_context(tc.tile_pool(name="xb", bufs=2))
        qkv_psum = ctx.enter_context(tc.tile_pool(name="qkvps", bufs=4, space="PSUM"))
        qkv_pool = ctx.enter_context(tc.tile_pool(name="qkv", bufs=2))
        tmp_pool = ctx.enter_context(tc.tile_pool(name="tmp", bufs=2))
        sm_pool = ctx.enter_context(tc.tile_pool(name="sm", bufs=2))
        outT_pool = ctx.enter_context(tc.tile_pool(name="outT", bufs=1))
        ot_psum = ctx.enter_context(tc.tile_pool(name="otps", bufs=1, space="PSUM"))
        od_pool = ctx.enter_context(tc.tile_pool(name="od", bufs=6))
        o_psum = ctx.enter_context(tc.tile_pool(name="ops", bufs=3, space="PSUM"))

        first_head = [True]

        def emit_head(b):
            """DMA-in + fp8 DoubleRow QKV projections (PE-heavy)."""
            xhi = [x8_pool.tile([128, 2 * TOK], fp8, name="xhi", tag="x8")
                   for _ in range(NPAIR)]
            xlo = [x8_pool.tile([128, 2 * TOK], fp8, name="xlo", tag="x8")
                   for _ in range(NPAIR)]
            for i in range(NPAIR):
                src = AP(xhi_d, (b * C + 2 * i * 128) * TOK,
                         [[TOK, 128], [128 * TOK, 2], [1, TOK]])
                nc.sync.dma_start(
                    ap(xhi[i][:], 0, [[2 * TOK, 128], [TOK, 2], [1, TOK]]), src)
                src = AP(xlo_d, (b * C + 2 * i * 128) * TOK,
                         [[TOK, 128], [128 * TOK, 2], [1, TOK]])
                nc.sync.dma_start(
                    ap(xlo[i][:], 0, [[2 * TOK, 128], [TOK, 2], [1, TOK]]), src)
            if first_head[0]:
                first_head[0] = False
                load_weights()

            q_all = qkv_pool.tile([128, 6 * D], bf16, tag="q")
            k_all = qkv_pool.tile([128, 6 * D], bf16, tag="k")
            v_all = qkv_pool.tile([128, 6 * D], bf16, tag="v")
            qkv_all = (q_all, k_all, v_all)
            ones_lhs = ap(ones8_sb[:], 0, [[256, 1], [128, 2], [1, 128]])
            bq_rhs = ap(bq8_sb[:], 0, [[1024, 1], [512, 2], [1, 512]])
            inv = 1.0 / WS
            # per (proj, p): finish the whole accumulation chain before the
            # next one starts, so the PSUM->SBUF copy overlaps the next
            # projection's matmuls and the PSUM ring recycles early.  Phase
            # order k, v, q: the attention stage needs ALL k (and all v for
            # att*v) but only q[p] per qk step, so emitting q last lets the
            # DVE attention chain start while the q-phase is still running.
            for j in (1, 2, 0):
                for p in range(P):
                    psj = qkv_psum.tile([128, D], f32, name="ps", tag="qkvps",
                                        bufs=3)
                    if j == 0:
                        # q bias via K=1 fp8 DoubleRow ones-row matmul
                        nc.tensor.matmul(psj[:], lhsT=ones_lhs, rhs=bq_rhs,
                                         start=True, stop=False, perf_mode=DR)
                    for i in range(NPAIR):
                        lhsT = ap(xhi[i][:], p * 128,
                                  [[2 * TOK, 128], [TOK, 2], [1, 128]])
                        nc.tensor.matmul(
                            psj[:], lhsT=lhsT,
                            rhs=ap(whi_sb[i][:], j * D,
                                   [[2 * 3 * D, 128], [3 * D, 2], [1, D]]),
                            start=(i == 0 and j > 0), stop=False, perf_mode=DR)
                        nc.tensor.matmul(
                            psj[:], lhsT=lhsT,
                            rhs=ap(wlo_sb[i][:], j * D,
                                   [[2 * 3 * D, 128], [3 * D, 2], [1, D]]),
                            start=False, stop=False, perf_mode=DR)
                    for i in range(NPAIR):
                        lhsT = ap(xlo[i][:], p * 128,
                                  [[2 * TOK, 128], [TOK, 2], [1, 128]])
                        nc.tensor.matmul(
                            psj[:], lhsT=lhsT,
                            rhs=ap(whi_sb[i][:], j * D,
                                   [[2 * 3 * D, 128], [3 * D, 2], [1, D]]),
                            start=False, stop=(i == NPAIR - 1), perf_mode=DR)
                    nc.scalar.activation(
                        qkv_all[j][:, p * D:(p + 1) * D], psj[:], COPY,
                        scale=inv)
            return q_all, k_all, v_all

        def emit_attn(b, q_all, k_all, v_all):
            """Attention stage, software-pipelined over p: qk(p) runs while
            softmax(p-1) and att*v(p-2) drain, keeping the DVE dense despite
            the ScalarE hops (exp, broadcast-expand)."""
            qk = sm_pool.tile([128, P * G * P], f32, tag="qk")   # [128, 288]
            attn = sm_pool.tile([128, 288], bf16, tag="attn")
            ssum = sm_pool.tile([128, 48], f32, tag="ssum")
            od = []

            def qk_ops(p):
                for h in range(2):  # q' half
                    tmp = tmp_pool.tile([128, 3 * D], bf16, tag="tmp")
                    in0 = ap(q_all[:], p * D,
                             [[6 * D, 128], [0, 3], [FD, G], [1, FD]])
                    in1 = ap(k_all[:], h * 3 * D,
                             [[6 * D, 128], [D, 3], [FD, G], [1, FD]])
                    o3 = ap(tmp[:], 0, [[3 * D, 128], [D, 3], [FD, G], [1, FD]])
                    nc.vector.tensor_tensor(o3, in0, in1, op=MULT)
                    # log2 add-tree over f (bf16 2x-mode TT beats 1x reduce)
                    w = FD
                    while w > 2:
                        w //= 2
                        nc.vector.tensor_tensor(
                            ap(tmp[:], 0, [[3 * D, 128], [FD, 24], [1, w]]),
                            ap(tmp[:], 0, [[3 * D, 128], [FD, 24], [1, w]]),
                            ap(tmp[:], w, [[3 * D, 128], [FD, 24], [1, w]]),
                            op=ADD)
                    nc.vector.tensor_tensor(
                        ap(qk[:], p * 48 + h * 3, [[288, 128], [1, 3], [6, G]]),
                        ap(tmp[:], 0, [[3 * D, 128], [D, 3], [FD, G]]),
                        ap(tmp[:], 1, [[3 * D, 128], [D, 3], [FD, G]]),
                        op=ADD)

            def sm_ops(p):
                # per-p softmax over q'; no max-subtraction (logits bounded)
                nc.scalar.activation(qk[:, p * 48:(p + 1) * 48],
                                     qk[:, p * 48:(p + 1) * 48], EXP, scale=0.125)
                nc.vector.tensor_reduce(
                    ssum[:, p * G:(p + 1) * G],
                    ap(qk[:], p * 48, [[288, 128], [6, G], [1, 6]]),
                    axis=AX, op=ADD)
                nc.vector.reciprocal(ssum[:, p * G:(p + 1) * G],
                                     ssum[:, p * G:(p + 1) * G])
                nc.vector.tensor_tensor(
                    ap(attn[:], p * 48, [[288, 128], [6, G], [1, 6]]),
                    ap(qk[:], p * 48, [[288, 128], [6, G], [1, 6]]),
                    ap(ssum[:], p * G, [[48, 128], [1, G], [0, 6]]), op=MULT)

            def av_ops(p):
                # out^T[n,(g,f)] = sum_q' attn[n,(p,g,q')] * v[n,(q',g,f)]
                outT = outT_pool.tile([128, D], bf16, name="outT", tag="outT")
                for h in range(2):  # g half
                    tmp2 = tmp_pool.tile([128, 3 * D], bf16, tag="tmp")
                    a0 = ap(attn[:], p * 48 + h * 4 * P,
                            [[288, 128], [1, 6], [6, 4], [0, FD]])
                    v0 = ap(v_all[:], h * 4 * FD,
                            [[6 * D, 128], [D, 6], [FD, 4], [1, FD]])
                    t0 = ap(tmp2[:], 0, [[3 * D, 128], [256, 6], [FD, 4], [1, FD]])
                    # broadcast-expand attn over f on ScalarE (otherwise the
                    # step-0 input AP forces the DVE multiply into 1x mode)
                    nc.scalar.activation(t0, a0, COPY)
                    nc.vector.tensor_tensor(tmp2[:], tmp2[:], v0, op=MULT)
                    # add-tree over q' (6 planes of 256)
                    nc.vector.tensor_tensor(
                        tmp2[:, 0:768], tmp2[:, 0:768], tmp2[:, 768:1536], op=ADD)
                    nc.vector.tensor_tensor(
                        tmp2[:, 0:256], tmp2[:, 0:256], tmp2[:, 512:768], op=ADD)
                    nc.vector.tensor_tensor(
                        ap(outT[:], h * 4 * FD, [[D, 128], [1, 256]]),
                        tmp2[:, 0:256], tmp2[:, 256:512], op=ADD)

                ps = ot_psum.tile([128, D], bf16, name="ps", tag="tps", bufs=2)
                for dc in range(4):
                    nc.tensor.transpose(
                        ps[:, dc * 128:(dc + 1) * 128],
                        outT[:, dc * 128:(dc + 1) * 128],
                        identb[:])
                od_p = od_pool.tile([128, D], bf16, name="od", tag="od")
                nc.scalar.activation(od_p[:], ps[:], COPY)
                od.append(od_p)

            for pp in range(P + 2):
                if pp < P:
                    qk_ops(pp)
                if 1 <= pp <= P:
                    sm_ops(pp - 1)
                if pp >= 2:
                    av_ops(pp - 2)
            return od

        def emit_out(b, od):
            """bf16 output projection + residual into freshly-read xb tiles
            (re-read from DRAM so x tiles don't pin the pipeline) + store."""
            r0 = b * NB
            for co in range(4):
                xb = xb_pool.tile([128, QF], bf16, name="xb", tag="xb")
                nc.gpsimd.dma_start(
                    xb[:], xb_d.ap()[r0:r0 + NB, co * QF:(co + 1) * QF])
                for p in range(P):
                    pso = o_psum.tile([128, D], f32, name="pso", tag="ops")
                    for dc in range(4):
                        nc.tensor.matmul(
                            pso[:], lhsT=od[p][:, dc * 128:(dc + 1) * 128],
                            rhs=woT_sb[dc][:, co * D:(co + 1) * D],
                            start=(dc == 0), stop=(dc == 3))
                    xsl = ap(xb[:], p, [[QF, 128], [P, D]])
                    # residual add on DVE (GPSIMD cannot read PSUM); lands in
                    # the out window where the DVE is otherwise idle
                    nc.vector.tensor_tensor(xsl, pso[:], xsl, op=ADD)
                nc.gpsimd.dma_start(
                    out_d.ap()[r0:r0 + NB, co * QF:(co + 1) * QF], xb[:])

        if reps == 0:
            # timing-baseline null program: same I/O tensors, trivial work
            z = x8_pool.tile([128, 2 * TOK], fp8, name="xhi", tag="x8")
            nc.sync.dma_start(z[:, 0:64], AP(xhi_d, 0, [[TOK, 128], [1, 64]]))
            zf = xb_pool.tile([128, QF], bf16, name="xb", tag="xb")
            nc.gpsimd.memset(zf[:, 0:64], 0)
            nc.sync.dma_start(
                AP(out_d, 0, [[CP, 128], [1, 64]]), zf[:, 0:64])
            load_weights()
            nb_total = 0
        else:
            nb_total = nblocks * reps

        # 2-stage software pipeline: head (PE projections) and
        # attention+output trailing by one block.
        hcarry = None
        for i in range(nb_total + 1):
            nxt_h = None
            if i < nb_total:
                nxt_h = (i % nblocks, emit_head(i % nblocks))
            if hcarry is not None:
                hb, h = hcarry
                od = emit_attn(hb, *h)
                emit_out(hb, od)
            hcarry = nxt_h

    _dedupe_ldweights(nc, mybir)
    nc.compile()
    return nc


def _dedupe_ldweights(nc, mybir):
    """Drop InstLdweights whose weights AP is identical to the previous one
    on the PE stream (no intervening transpose, which reloads the array).
    The scheduler places same-lhsT matmuls back to back after the loop
    reordering, so this removes most of the PE-sequencer LDW dispatch cost.
    Waits/updates on a dropped LDW are merged into the following matmul's
    sync_info (multi-wait is legal pre-compile; generate_event_semaphores
    splits them later)."""

    def apkey(a):
        return (str(a.memref), str(a.offset), str(a.ap), str(a.dtype))

    for blk in nc.m.functions[0].blocks:
        insts = blk.instructions
        last = None
        drop = set()
        pending_sync = []
        for idx, ins in enumerate(insts):
            nm = type(ins).__name__
            if nm == "InstLdweights":
                key = (apkey(ins.ins[0]), str(ins.perf_mode),
                       str(ins.is_transpose), str(ins.tile_position))
                if key == last:
                    drop.add(idx)
                    if ins.sync_info is not None:
                        pending_sync.append(ins.sync_info)
                last = key
            elif nm == "InstMatmult":
                if getattr(ins, "is_transpose", False):
                    last = None
                if pending_sync:
                    si = ins.sync_info
                    if si is None:
                        si = mybir.SyncInfo(on_wait=[], on_update=[])
                    for extra in pending_sync:
                        si.on_wait = list(si.on_wait) + list(extra.on_wait)
                        si.on_update = list(si.on_update) + list(extra.on_update)
                    ins.sync_info = si
                    pending_sync = []
        if drop:
            assert not pending_sync
            keep = [i for idx, i in enumerate(insts) if idx not in drop]
            del insts[:]
            insts.extend(keep)


def get_program(ns, reps=1):
    key = (ns, reps)
    if key not in _CACHE:
        _CACHE[key] = _build(ns, reps)
    return _CACHE[key]


def _host_prep(inputs):
    """Host-side weight/bias prep (shared across cores)."""
    bf = ml_dtypes.bfloat16
    e4 = ml_dtypes.float8_e4m3
    wq = np.asarray(inputs["wq"], np.float32)
    wk = np.asarray(inputs["wk"], np.float32)
    wv = np.asarray(inputs["wv"], np.float32)
    wo = np.asarray(inputs["wo"], np.float32)
    wT64 = np.ascontiguousarray(
        np.concatenate([wq.T, wk.T, wv.T], axis=1)) * np.float32(WS)  # [C, 3D]
    whi = wT64.astype(e4)
    wlo = (wT64 - whi.astype(np.float32)).astype(e4)
    woT = np.ascontiguousarray(wo.T).astype(bf)                        # [D, C]
    # k-bias is softmax-invariant (adds a row-constant to the logits);
    # v-bias passes through attention unchanged (sum(attn)==1) so it folds
    # into the output-projection bias: bo_eff = bo + wo @ bv.
    bq = np.asarray(inputs["bq"], np.float32)
    bo_eff = (np.asarray(inputs["bo"], np.float32)
              + np.asarray(wo, np.float64) @ np.asarray(inputs["bv"], np.float64)
              ).astype(np.float32)
    bq8 = np.zeros((1, 1024), e4)
    bq8[0, 0:D] = (WS * bq).astype(e4)
    ones8 = np.zeros((1, 256), e4)
    ones8[0, 0:128] = np.float32(1.0)
    return whi, wlo, woT, ones8, bq8, bo_eff


def _host_x_prep(xs, bo_eff):
    """Per-core x prep: c-major fp8 hi/lo planes + n-major bf16 residual."""
    bf = ml_dtypes.bfloat16
    e4 = ml_dtypes.float8_e4m3
    ns = xs.shape[0]
    nb = ns // NB
    # [ns, C, P] -> [nb, C, P, NB] -> [nb*C, TOK]
    xT = np.ascontiguousarray(
        xs.reshape(nb, NB, C, P).transpose(0, 2, 3, 1)).reshape(nb * C, TOK)
    xhi = xT.astype(e4)
    xlo = (xT - xhi.astype(np.float32)).astype(e4)
    xb = (xs + bo_eff[None, :, None]).astype(bf).reshape(ns, CP)
    return xhi, xlo, np.ascontiguousarray(xb)


def kernel(**inputs):
    from concourse.bass_utils import run_bass_kernel_spmd

    x = np.asarray(inputs["parts_feat"], np.float32)
    n_total = x.shape[0]
    xs_all = x.reshape(n_total, C, P)
    ns = n_total // NCORES
    whi, wlo, woT, ones8, bq8, bo_eff = _host_prep(inputs)

    nc = get_program(ns)
    in_maps = []
    for i in range(NCORES):
        xhi, xlo, xb = _host_x_prep(xs_all[i * ns:(i + 1) * ns], bo_eff)
        in_maps.append({
            "xhi": xhi, "xlo": xlo, "xb": xb,
            "whi": whi, "wlo": wlo, "woT": woT,
            "ones8": ones8, "bq8": bq8,
        })
    res = run_bass_kernel_spmd(nc, in_maps, core_ids=list(range(NCORES)))
    out = np.concatenate([r["out"] for r in res.results], axis=0)
    # reference() squeezes the trailing singleton: output is [N, C, P]
    return out.astype(np.float32).reshape(n_total, C, P)


# revision 13
# speedup vs baseline: 1.0099x; 1.0099x over previous
"""Trainium2 Bass kernel for nn_AttPCB (grouped 6-token attention block).

Math (per sample n):
  x   = parts_feat[n,:,:,0]                      # [C=2048, P=6]
  q/k/v = W x + b                                # [D=512, 6]
  per group g (8 groups of 64 channels):
    qk = (Qg^T Kg) / 8 ; attn = softmax(qk, -1)  # [6, 6]
    out_g = Vg @ attn^T                          # [64, 6]
  o  = wo @ out + bo                             # [2048, 6]
  ret = x + o

Distribution: pure data parallel over N=4096 samples across 8 cores
(512 samples/core).  Weights are replicated.

Key design points (v2):
  * QKV projections run as fp8e4m3 DoubleRow matmuls (0.5 cyc/row) using a
    3-term hi/lo error split: x*w ~= x_hi*w_hi + x_hi*w_lo + x_lo*w_hi,
    which costs 0.75x the bf16 rows with near-bf16 accuracy (rel err
    ~3e-3 end to end).  w is pre-scaled by 64 on the host so its hi/lo fp8
    planes stay in the normal range; the PSUM->SBUF copy rescales by 1/64.
  * x is uploaded pre-transposed (c-major, block-major) as fp8 hi/lo
    planes, eliminating all PE transposes of x and their PSUM->SBUF hops.
  * The residual stream xb = bf16(x + bo_eff) is uploaded n-major; the
    output-projection bias is folded into it (with the bias-feedback
    corrections bq_eff/bo_eff computed on the host), the k bias is
    softmax-invariant and dropped, and the v bias folds into bo_eff.
    q bias enters as a K=1 fp8 DoubleRow ones-row matmul.
  * Attention math stays bf16 on DVE (2x mode): qk products + log2
    add-trees, per-p softmax without max-subtraction, attn*v with a
    ScalarE broadcast-expand.  out^T is PE-transposed to d-major for the
    bf16 output projection; the residual add reads pso straight from PSUM
    and accumulates into the xb tiles, which are then stored as bf16.
  * I/O is bf16/fp8 end to end (~37.8 MB/core vs 75.5 fp32), with the
    final bf16->fp32 widening done on the host.
LDWEIGHTS deduplication runs as a post-pass before compile.
"""

import numpy as np
import ml_dtypes

N_FULL = 4096
C = 2048
P = 6
D = 512
G = 8
FD = 64
NCORES = 8
NB = 128          # samples per block
CP = C * P        # 12288
QF = CP // 4      # free elems per c-quarter tile (3072)
TOK = NB * P      # tokens per block (768)
NPAIR = 8         # 128-channel chunk pairs in C
WS = 64.0         # host-side scale on the QKV weight planes

_CACHE = {}


def _build(ns, reps=1):
    """Build the Bass/Tile program for one core processing ns samples."""
    from contextlib import ExitStack

    import concourse.bass as bass
    import concourse.tile as tile
    import concourse.mybir as mybir
    from concourse import bacc
    from concourse.bass_types import AP
    from concourse.masks import make_identity

    f32 = mybir.dt.float32
    bf16 = mybir.dt.bfloat16
    fp8 = mybir.dt.float8e4
    MULT = mybir.AluOpType.mult
    ADD = mybir.AluOpType.add
    AX = mybir.AxisListType.X
    COPY = mybir.ActivationFunctionType.Copy
    EXP = mybir.ActivationFunctionType.Exp
    DR = mybir.MatmulPerfMode.DoubleRow

    assert ns % NB == 0
    nblocks = ns // NB

    nc = bacc.Bacc("TRN2", target_bir_lowering=False, debug=False)

    xhi_d = nc.dram_tensor("xhi", [nblocks * C, TOK], fp8, kind="ExternalInput")
    xlo_d = nc.dram_tensor("xlo", [nblocks * C, TOK], fp8, kind="ExternalInput")
    xb_d = nc.dram_tensor("xb", [ns, CP], bf16, kind="ExternalInput")
    whi_d = nc.dram_tensor("whi", [C, 3 * D], fp8, kind="ExternalInput")
    wlo_d = nc.dram_tensor("wlo", [C, 3 * D], fp8, kind="ExternalInput")
    woT_d = nc.dram_tensor("woT", [D, C], bf16, kind="ExternalInput")
    ones8_d = nc.dram_tensor("ones8", [1, 256], fp8, kind="ExternalInput")
    bq8_d = nc.dram_tensor("bq8", [1, 1024], fp8, kind="ExternalInput")
    out_d = nc.dram_tensor("out", [ns, CP], bf16, kind="ExternalOutput")

    def ap(tile_ap, off, dims):
        """Custom access pattern into a tile: dims = [[step,count],...]."""
        return AP(tile_ap.tensor, tile_ap.offset + off, dims)

    with ExitStack() as ctx:
        tc = ctx.enter_context(tile.TileContext(nc))

        # ---- persistent weights / constants ----
        wpool = ctx.enter_context(tc.tile_pool(name="w8", bufs=16))
        wopool = ctx.enter_context(tc.tile_pool(name="woT", bufs=4))
        cpool = ctx.enter_context(tc.tile_pool(name="const", bufs=1))

        # fp8 weight pair tiles: [128, (chunk 2) x (proj 3 x 512)]
        whi_sb = [wpool.tile([128, 2 * 3 * D], fp8, name="whi", tag="whi")
                  for _ in range(NPAIR)]
        wlo_sb = [wpool.tile([128, 2 * 3 * D], fp8, name="wlo", tag="wlo")
                  for _ in range(NPAIR)]
        woT_sb = [wopool.tile([128, C], bf16, name="woTsb", tag="woT")
                  for _ in range(4)]

        def load_weights():
            # emitted after block 0's x DMAs so x streams in first
            for i in range(NPAIR):
                src = AP(whi_d, (2 * i * 128) * 3 * D,
                         [[3 * D, 128], [128 * 3 * D, 2], [1, 3 * D]])
                nc.sync.dma_start(
                    ap(whi_sb[i][:], 0, [[2 * 3 * D, 128], [3 * D, 2], [1, 3 * D]]),
                    src)
                src = AP(wlo_d, (2 * i * 128) * 3 * D,
                         [[3 * D, 128], [128 * 3 * D, 2], [1, 3 * D]])
                nc.sync.dma_start(
                    ap(wlo_sb[i][:], 0, [[2 * 3 * D, 128], [3 * D, 2], [1, 3 * D]]),
                    src)
            for dc in range(4):
                nc.sync.dma_start(woT_sb[dc][:], woT_d.ap()[dc * 128:(dc + 1) * 128, :])

        ones8_sb = cpool.tile([1, 256], fp8, tag="ones8")
        bq8_sb = cpool.tile([1, 1024], fp8, tag="bq8")
        identb = cpool.tile([128, 128], bf16, tag="identb")
        nc.sync.dma_start(ones8_sb[:], ones8_d.ap()[:, :])
        nc.sync.dma_start(bq8_sb[:], bq8_d.ap()[:, :])
        make_identity(nc, identb[:])

        # ---- per-block pools ----
        x8_pool = ctx.enter_context(tc.tile_pool(name="x8", bufs=19))
        xb_pool = ctx.enter_context(tc.tile_pool(name="xb", bufs=2))
        qkv_psum = ctx.enter_context(tc.tile_pool(name="qkvps", bufs=4, space="PSUM"))
        qkv_pool = ctx.enter_context(tc.tile_pool(name="qkv", bufs=2))
        tmp_pool = ctx.enter_context(tc.tile_pool(name="tmp", bufs=2))
        sm_pool = ctx.enter_context(tc.tile_pool(name="sm", bufs=2))
        outT_pool = ctx.enter_context(tc.tile_pool(name="outT", bufs=1))
        ot_psum = ctx.enter_context(tc.tile_pool(name="otps", bufs=1, space="PSUM"))
        od_pool = ctx.enter_context(tc.tile_pool(name="od", bufs=6))
        o_psum = ctx.enter_context(tc.tile_pool(name="ops", bufs=3, space="PSUM"))

        first_head = [True]

        def emit_head(b):
            """DMA-in + fp8 DoubleRow QKV projections (PE-heavy)."""
            xhi = [x8_pool.tile([128, 2 * TOK], fp8, name="xhi", tag="x8")
                   for _ in range(NPAIR)]
            xlo = [x8_pool.tile([128, 2 * TOK], fp8, name="xlo", tag="x8")
                   for _ in range(NPAIR)]
            for i in range(NPAIR):
                src = AP(xhi_d, (b * C + 2 * i * 128) * TOK,
                         [[TOK, 128], [128 * TOK, 2], [1, TOK]])
                nc.sync.dma_start(
                    ap(xhi[i][:], 0, [[2 * TOK, 128], [TOK, 2], [1, TOK]]), src)
                src = AP(xlo_d, (b * C + 2 * i * 128) * TOK,
                         [[TOK, 128], [128 * TOK, 2], [1, TOK]])
                nc.sync.dma_start(
                    ap(xlo[i][:], 0, [[2 * TOK, 128], [TOK, 2], [1, TOK]]), src)
            if first_head[0]:
                first_head[0] = False
                load_weights()

            q_all = qkv_pool.tile([128, 6 * D], bf16, tag="q")
            k_all = qkv_pool.tile([128, 6 * D], bf16, tag="k")
            v_all = qkv_pool.tile([128, 6 * D], bf16, tag="v")
            qkv_all = (q_all, k_all, v_all)
            ones_lhs = ap(ones8_sb[:], 0, [[256, 1], [128, 2], [1, 128]])
            bq_rhs = ap(bq8_sb[:], 0, [[1024, 1], [512, 2], [1, 512]])
            inv = 1.0 / WS
            # per (proj, p): finish the whole accumulation chain before the
            # next one starts, so the PSUM->SBUF copy overlaps the next
            # projection's matmuls and the PSUM ring recycles early.  Phase
            # order k, v, q: the attention stage needs ALL k (and all v for
            # att*v) but only q[p] per qk step, so emitting q last lets the
            # DVE attention chain start while the q-phase is still running.
            for j in (1, 2, 0):
                for p in range(P):
                    psj = qkv_psum.tile([128, D], f32, name="ps", tag="qkvps",
                                        bufs=3)
                    if j == 0:
                        # q bias via K=1 fp8 DoubleRow ones-row matmul
                        nc.tensor.matmul(psj[:], lhsT=ones_lhs, rhs=bq_rhs,
                                         start=True, stop=False, perf_mode=DR)
                    for i in range(NPAIR):
                        lhsT = ap(xhi[i][:], p * 128,
                                  [[2 * TOK, 128], [TOK, 2], [1, 128]])
                        nc.tensor.matmul(
                            psj[:], lhsT=lhsT,
                            rhs=ap(whi_sb[i][:], j * D,
                                   [[2 * 3 * D, 128], [3 * D, 2], [1, D]]),
                            start=(i == 0 and j > 0), stop=False, perf_mode=DR)
                        nc.tensor.matmul(
                            psj[:], lhsT=lhsT,
                            rhs=ap(wlo_sb[i][:], j * D,
                                   [[2 * 3 * D, 128], [3 * D, 2], [1, D]]),
                            start=False, stop=False, perf_mode=DR)
                    for i in range(NPAIR):
                        lhsT = ap(xlo[i][:], p * 128,
                                  [[2 * TOK, 128], [TOK, 2], [1, 128]])
                        nc.tensor.matmul(
                            psj[:], lhsT=lhsT,
                            rhs=ap(whi_sb[i][:], j * D,
                                   [[2 * 3 * D, 128], [3 * D, 2], [1, D]]),
                            start=False, stop=(i == NPAIR - 1), perf_mode=DR)
                    nc.scalar.activation(
                        qkv_all[j][:, p * D:(p + 1) * D], psj[:], COPY,
                        scale=inv)
            return q_all, k_all, v_all

        def emit_attn(b, q_all, k_all, v_all):
            """Attention stage, software-pipelined over p: qk(p) runs while
            softmax(p-1) and att*v(p-2) drain, keeping the DVE dense despite
            the ScalarE hops (exp, broadcast-expand)."""
            qk = sm_pool.tile([128, P * G * P], f32, tag="qk")   # [128, 288]
            attn = sm_pool.tile([128, 288], bf16, tag="attn")
            ssum = sm_pool.tile([128, 48], f32, tag="ssum")
            od = []

            def qk_ops(p):
                for h in range(2):  # q' half
                    tmp = tmp_pool.tile([128, 3 * D], bf16, tag="tmp")
                    in0 = ap(q_all[:], p * D,
                             [[6 * D, 128], [0, 3], [FD, G], [1, FD]])
                    in1 = ap(k_all[:], h * 3 * D,
                             [[6 * D, 128], [D, 3], [FD, G], [1, FD]])
                    o3 = ap(tmp[:], 0, [[3 * D, 128], [D, 3], [FD, G], [1, FD]])
                    nc.vector.tensor_tensor(o3, in0, in1, op=MULT)
                    # log2 add-tree over f (bf16 2x-mode TT beats 1x reduce)
                    w = FD
                    while w > 2:
                        w //= 2
                        nc.vector.tensor_tensor(
                            ap(tmp[:], 0, [[3 * D, 128], [FD, 24], [1, w]]),
                            ap(tmp[:], 0, [[3 * D, 128], [FD, 24], [1, w]]),
                            ap(tmp[:], w, [[3 * D, 128], [FD, 24], [1, w]]),
                            op=ADD)
                    nc.vector.tensor_tensor(
                        ap(qk[:], p * 48 + h * 3, [[288, 128], [1, 3], [6, G]]),
                        ap(tmp[:], 0, [[3 * D, 128], [D, 3], [FD, G]]),
                        ap(tmp[:], 1, [[3 * D, 128], [D, 3], [FD, G]]),
                        op=ADD)

            def sm_ops(p):
                # per-p softmax over q'; no max-subtraction (logits bounded)
                nc.scalar.activation(qk[:, p * 48:(p + 1) * 48],
                                     qk[:, p * 48:(p + 1) * 48], EXP, scale=0.125)
                nc.vector.tensor_reduce(
                    ssum[:, p * G:(p + 1) * G],
                    ap(qk[:], p * 48, [[288, 128], [6, G], [1, 6]]),
                    axis=AX, op=ADD)
                nc.vector.reciprocal(ssum[:, p * G:(p + 1) * G],
                                     ssum[:, p * G:(p + 1) * G])
                nc.vector.tensor_tensor(
                    ap(attn[:], p * 48, [[288, 128], [6, G], [1, 6]]),
                    ap(qk[:], p * 48, [[288, 128], [6, G], [1, 6]]),
                    ap(ssum[:], p * G, [[48, 128], [1, G], [0, 6]]), op=MULT)

            def av_ops(p):
                # out^T[n,(g,f)] = sum_q' attn[n,(p,g,q')] * v[n,(q',g,f)]
                outT = outT_pool.tile([128, D], bf16, name="outT", tag="outT")
                for h in range(2):  # g half
                    tmp2 = tmp_pool.tile([128, 3 * D], bf16, tag="tmp")
                    a0 = ap(attn[:], p * 48 + h * 4 * P,
                            [[288, 128], [1, 6], [6, 4], [0, FD]])
                    v0 = ap(v_all[:], h * 4 * FD,
                            [[6 * D, 128], [D, 6], [FD, 4], [1, FD]])
                    t0 = ap(tmp2[:], 0, [[3 * D, 128], [256, 6], [FD, 4], [1, FD]])
                    # broadcast-expand attn over f on ScalarE (otherwise the
                    # step-0 input AP forces the DVE multiply into 1x mode)
                    nc.scalar.activation(t0, a0, COPY)
                    nc.vector.tensor_tensor(tmp2[:], tmp2[:], v0, op=MULT)
                    # add-tree over q' (6 planes of 256)
                    nc.vector.tensor_tensor(
                        tmp2[:, 0:768], tmp2[:, 0:768], tmp2[:, 768:1536], op=ADD)
                    nc.vector.tensor_tensor(
                        tmp2[:, 0:256], tmp2[:, 0:256], tmp2[:, 512:768], op=ADD)
                    nc.vector.tensor_tensor(
                        ap(outT[:], h * 4 * FD, [[D, 128], [1, 256]]),
                        tmp2[:, 0:256], tmp2[:, 256:512], op=ADD)

                ps = ot_psum.tile([128, D], bf16, name="ps", tag="tps", bufs=2)
                for dc in range(4):
                    nc.tensor.transpose(
                        ps[:, dc * 128:(dc + 1) * 128],
                        outT[:, dc * 128:(dc + 1) * 128],
                        identb[:])
                od_p = od_pool.tile([128, D], bf16, name="od", tag="od")
                nc.scalar.activation(od_p[:], ps[:], COPY)
                od.append(od_p)

            for pp in range(P + 2):
                if pp < P:
                    qk_ops(pp)
                if 1 <= pp <= P:
                    sm_ops(pp - 1)
                if pp >= 2:
                    av_ops(pp - 2)
            return od

        def emit_out(b, od):
            """bf16 output projection + residual into freshly-read xb tiles
            (re-read from DRAM so x tiles don't pin the pipeline) + store."""
            r0 = b * NB
            for co in range(4):
                xb = xb_pool.tile([128, QF], bf16, name="xb", tag="xb")
                nc.sync.dma_start(
                    xb[:], xb_d.ap()[r0:r0 + NB, co * QF:(co + 1) * QF])
                for p in range(P):
                    pso = o_psum.tile([128, D], f32, name="pso", tag="ops")
                    for dc in range(4):
                        nc.tensor.matmul(
                            pso[:], lhsT=od[p][:, dc * 128:(dc + 1) * 128],
                            rhs=woT_sb[dc][:, co * D:(co + 1) * D],
                            start=(dc == 0), stop=(dc == 3))
                    xsl = ap(xb[:], p, [[QF, 128], [P, D]])
                    # residual add on DVE (GPSIMD cannot read PSUM); lands in
                    # the out window where the DVE is otherwise idle
                    nc.vector.tensor_tensor(xsl, pso[:], xsl, op=ADD)
                nc.sync.dma_start(
                    out_d.ap()[r0:r0 + NB, co * QF:(co + 1) * QF], xb[:])

        if reps == 0:
            # timing-baseline null program: same I/O tensors, trivial work
            z = x8_pool.tile([128, 2 * TOK], fp8, name="xhi", tag="x8")
            nc.sync.dma_start(z[:, 0:64], AP(xhi_d, 0, [[TOK, 128], [1, 64]]))
            zf = xb_pool.tile([128, QF], bf16, name="xb", tag="xb")
            nc.gpsimd.memset(zf[:, 0:64], 0)
            nc.sync.dma_start(
                AP(out_d, 0, [[CP, 128], [1, 64]]), zf[:, 0:64])
            load_weights()
            nb_total = 0
        else:
            nb_total = nblocks * reps

        # 2-stage software pipeline: head (PE projections) and
        # attention+output trailing by one block.
        hcarry = None
        for i in range(nb_total + 1):
            nxt_h = None
            if i < nb_total:
                nxt_h = (i % nblocks, emit_head(i % nblocks))
            if hcarry is not None:
                hb, h = hcarry
                od = emit_attn(hb, *h)
                emit_out(hb, od)
            hcarry = nxt_h

    _dedupe_ldweights(nc, mybir)
    nc.compile()
    return nc


def _dedupe_ldweights(nc, mybir):
    """Drop InstLdweights whose weights AP is identical to the previous one
    on the PE stream (no intervening transpose, which reloads the array).
    The scheduler places same-lhsT matmuls back to back after the loop
    reordering, so this removes most of the PE-sequencer LDW dispatch cost.
    Waits/updates on a dropped LDW are merged into the following matmul's
    sync_info (multi-wait is legal pre-compile; generate_event_semaphores
    splits them later)."""

    def apkey(a):
        return (str(a.memref), str(a.offset), str(a.ap), str(a.dtype))

    for blk in nc.m.functions[0].blocks:
        insts = blk.instructions
        last = None
        drop = set()
        pending_sync = []
        for idx, ins in enumerate(insts):
            nm = type(ins).__name__
            if nm == "InstLdweights":
                key = (apkey(ins.ins[0]), str(ins.perf_mode),
                       str(ins.is_transpose), str(ins.tile_position))
                if key == last:
                    drop.add(idx)
                    if ins.sync_info is not None:
                        pending_sync.append(ins.sync_info)
                last = key
            elif nm == "InstMatmult":
                if getattr(ins, "is_transpose", False):
                    last = None
                if pending_sync:
                    si = ins.sync_info
                    if si is None:
                        si = mybir.SyncInfo(on_wait=[], on_update=[])
                    for extra in pending_sync:
                        si.on_wait = list(si.on_wait) + list(extra.on_wait)
                        si.on_update = list(si.on_update) + list(extra.on_update)
                    ins.sync_info = si
                    pending_sync = []
        if drop:
            assert not pending_sync
            keep = [i for idx, i in enumerate(insts) if idx not in drop]
            del insts[:]
            insts.extend(keep)


def get_program(ns, reps=1):
    key = (ns, reps)
    if key not in _CACHE:
        _CACHE[key] = _build(ns, reps)
    return _CACHE[key]


def _host_prep(inputs):
    """Host-side weight/bias prep (shared across cores)."""
    bf = ml_dtypes.bfloat16
    e4 = ml_dtypes.float8_e4m3
    wq = np.asarray(inputs["wq"], np.float32)
    wk = np.asarray(inputs["wk"], np.float32)
    wv = np.asarray(inputs["wv"], np.float32)
    wo = np.asarray(inputs["wo"], np.float32)
    wT64 = np.ascontiguousarray(
        np.concatenate([wq.T, wk.T, wv.T], axis=1)) * np.float32(WS)  # [C, 3D]
    whi = wT64.astype(e4)
    wlo = (wT64 - whi.astype(np.float32)).astype(e4)
    woT = np.ascontiguousarray(wo.T).astype(bf)                        # [D, C]
    # k-bias is softmax-invariant (adds a row-constant to the logits);
    # v-bias passes through attention unchanged (sum(attn)==1) so it folds
    # into the output-projection bias: bo_eff = bo + wo @ bv.
    bq = np.asarray(inputs["bq"], np.float32)
    bo_eff = (np.asarray(inputs["bo"], np.float32)
              + np.asarray(wo, np.float64) @ np.asarray(inputs["bv"], np.float64)
              ).astype(np.float32)
    bq8 = np.zeros((1, 1024), e4)
    bq8[0, 0:D] = (WS * bq).astype(e4)
    ones8 = np.zeros((1, 256), e4)
    ones8[0, 0:128] = np.float32(1.0)
    return whi, wlo, woT, ones8, bq8, bo_eff


def _host_x_prep(xs, bo_eff):
    """Per-core x prep: c-major fp8 hi/lo planes + n-major bf16 residual."""
    bf = ml_dtypes.bfloat16
    e4 = ml_dtypes.float8_e4m3
    ns = xs.shape[0]
    nb = ns // NB
    # [ns, C, P] -> [nb, C, P, NB] -> [nb*C, TOK]
    xT = np.ascontiguousarray(
        xs.reshape(nb, NB, C, P).transpose(0, 2, 3, 1)).reshape(nb * C, TOK)
    xhi = xT.astype(e4)
    xlo = (xT - xhi.astype(np.float32)).astype(e4)
    xb = (xs + bo_eff[None, :, None]).astype(bf).reshape(ns, CP)
    return xhi, xlo, np.ascontiguousarray(xb)


def kernel(**inputs):
    from concourse.bass_utils import run_bass_kernel_spmd

    x = np.asarray(inputs["parts_feat"], np.float32)
    n_total = x.shape[0]
    xs_all = x.reshape(n_total, C, P)
    ns = n_total // NCORES
    whi, wlo, woT, ones8, bq8, bo_eff = _host_prep(inputs)

    nc = get_program(ns)
    in_maps = []
    for i in range(NCORES):
        xhi, xlo, xb = _host_x_prep(xs_all[i * ns:(i + 1) * ns], bo_eff)
        in_maps.append({
            "xhi": xhi, "xlo": xlo, "xb": xb,
            "whi": whi, "wlo": wlo, "woT": woT,
            "ones8": ones8, "bq8": bq8,
        })
    res = run_bass_kernel_spmd(nc, in_maps, core_ids=list(range(NCORES)))
    out = np.concatenate([r["out"] for r in res.results], axis=0)
    # reference() squeezes the trailing singleton: output is [N, C, P]
    return out.astype(np.float32).reshape(n_total, C, P)


# revision 14
# speedup vs baseline: 1.7312x; 1.7142x over previous
"""Trainium2 Bass kernel for nn_AttPCB (grouped 6-token attention block).

Math (per sample n):
  x   = parts_feat[n,:,:,0]                      # [C=2048, P=6]
  q/k/v = W x + b                                # [D=512, 6]
  per group g (8 groups of 64 channels):
    qk = (Qg^T Kg) / 8 ; attn = softmax(qk, -1)  # [6, 6]
    out_g = Vg @ attn^T                          # [64, 6]
  o  = wo @ out + bo                             # [2048, 6]
  ret = x + o

Distribution: pure data parallel over N=4096 samples across 8 cores
(512 samples/core).  Weights are replicated.

Key design points (v3):
  * x is uploaded pre-transposed (c-major, block-major) in bf16,
    eliminating all PE transposes of x and their PSUM->SBUF hops that the
    original in-kernel dataflow needed.  (fp8 DoubleRow was measured on
    this hardware to give ZERO matmul speedup over bf16 despite the cost
    model's 0.5 cyc/row, so the projections stay bf16.)
  * The residual stream xb = bf16(x + bo_eff) is uploaded n-major with
    the output-projection bias folded in; the k bias is softmax-invariant
    and dropped; the v bias folds into bo_eff = bo + wo @ bv; the q bias
    enters as a K=1 ones-row matmul.  No bias matmuls remain for bo.
  * QKV runs in proj-phases (k, v, q) with per-(proj,p) accumulation
    chains: the PSUM->SBUF copy of one chain overlaps the next chain's
    matmuls, and emitting q last lets the DVE attention chain (which
    needs all k and all v but only q[p] per step) start while the q-phase
    is still on the PE.
  * Attention math stays bf16 on DVE (2x mode): qk products + log2
    add-trees, per-p softmax without max-subtraction, attn*v with a
    ScalarE broadcast-expand.  out^T is PE-transposed to d-major for the
    bf16 output projection; the residual add reads pso straight from PSUM
    and accumulates into freshly-read xb tiles (quarter-major so each
    store leaves as soon as its residuals land).
  * I/O is bf16 end to end (~37.8 MB/core vs 75.5 fp32), with the final
    bf16->fp32 widening done on the host.
LDWEIGHTS deduplication runs as a post-pass before compile.
"""

import numpy as np
import ml_dtypes

N_FULL = 4096
C = 2048
P = 6
D = 512
G = 8
FD = 64
NCORES = 8
NB = 128          # samples per block
CP = C * P        # 12288
QF = CP // 4      # free elems per c-quarter tile (3072)
TOK = NB * P      # tokens per block (768)
NCH = 16          # 128-channel chunks in C

_CACHE = {}


def _build(ns, reps=1):
    """Build the Bass/Tile program for one core processing ns samples."""
    from contextlib import ExitStack

    import concourse.bass as bass
    import concourse.tile as tile
    import concourse.mybir as mybir
    from concourse import bacc
    from concourse.bass_types import AP
    from concourse.masks import make_identity

    f32 = mybir.dt.float32
    bf16 = mybir.dt.bfloat16
    MULT = mybir.AluOpType.mult
    ADD = mybir.AluOpType.add
    AX = mybir.AxisListType.X
    COPY = mybir.ActivationFunctionType.Copy
    EXP = mybir.ActivationFunctionType.Exp

    assert ns % NB == 0
    nblocks = ns // NB

    nc = bacc.Bacc("TRN2", target_bir_lowering=False, debug=False)

    xT_d = nc.dram_tensor("xT", [nblocks * C, TOK], bf16, kind="ExternalInput")
    xb_d = nc.dram_tensor("xb", [ns, CP], bf16, kind="ExternalInput")
    wT_d = nc.dram_tensor("wT", [C, 3 * D], bf16, kind="ExternalInput")
    woT_d = nc.dram_tensor("woT", [D, C], bf16, kind="ExternalInput")
    bq_d = nc.dram_tensor("bq", [1, D], bf16, kind="ExternalInput")
    out_d = nc.dram_tensor("out", [ns, CP], bf16, kind="ExternalOutput")

    def ap(tile_ap, off, dims):
        """Custom access pattern into a tile: dims = [[step,count],...]."""
        return AP(tile_ap.tensor, tile_ap.offset + off, dims)

    with ExitStack() as ctx:
        tc = ctx.enter_context(tile.TileContext(nc))

        # ---- persistent weights / constants ----
        wpool = ctx.enter_context(tc.tile_pool(name="wT", bufs=16))
        wopool = ctx.enter_context(tc.tile_pool(name="woT", bufs=4))
        cpool = ctx.enter_context(tc.tile_pool(name="const", bufs=1))

        wT_sb = [wpool.tile([128, 3 * D], bf16, name="wTsb", tag="wT")
                 for _ in range(NCH)]
        woT_sb = [wopool.tile([128, C], bf16, name="woTsb", tag="woT")
                  for _ in range(4)]

        def load_weights():
            for cc in range(NCH):
                nc.sync.dma_start(wT_sb[cc][:],
                                  wT_d.ap()[cc * 128:(cc + 1) * 128, :])
            for dc in range(4):
                nc.sync.dma_start(woT_sb[dc][:],
                                  woT_d.ap()[dc * 128:(dc + 1) * 128, :])

        bq_sb = cpool.tile([1, D], bf16, tag="bq")
        ones_sb = cpool.tile([1, 128], bf16, tag="ones")
        identb = cpool.tile([128, 128], bf16, tag="identb")
        nc.sync.dma_start(bq_sb[:], bq_d.ap()[:, :])
        nc.gpsimd.memset(ones_sb[:], 1.0)
        make_identity(nc, identb[:])

        # ---- per-block pools ----
        xT_pool = ctx.enter_context(tc.tile_pool(name="xT", bufs=20))
        xb_pool = ctx.enter_context(tc.tile_pool(name="xb", bufs=3))
        qkv_psum = ctx.enter_context(tc.tile_pool(name="qkvps", bufs=3,
                                                  space="PSUM"))
        qkv_pool = ctx.enter_context(tc.tile_pool(name="qkv", bufs=2))
        tmp_pool = ctx.enter_context(tc.tile_pool(name="tmp", bufs=3))
        sm_pool = ctx.enter_context(tc.tile_pool(name="sm", bufs=2))
        outT_pool = ctx.enter_context(tc.tile_pool(name="outT", bufs=1))
        ot_psum = ctx.enter_context(tc.tile_pool(name="otps", bufs=1,
                                                 space="PSUM"))
        od_pool = ctx.enter_context(tc.tile_pool(name="od", bufs=7))
        o_psum = ctx.enter_context(tc.tile_pool(name="ops", bufs=3,
                                                space="PSUM"))

        first_head = [True]

        def emit_head(b):
            """DMA-in + bf16 QKV projections (PE-heavy)."""
            xT = [xT_pool.tile([128, TOK], bf16, name="xT", tag="xT")
                  for _ in range(NCH)]
            if first_head[0]:
                # first block: interleave x and weight streams so the first
                # accumulation chains can start before everything landed
                first_head[0] = False
                for cc in range(NCH):
                    nc.sync.dma_start(
                        wT_sb[cc][:], wT_d.ap()[cc * 128:(cc + 1) * 128, :])
                    nc.sync.dma_start(
                        xT[cc][:],
                        xT_d.ap()[b * C + cc * 128:b * C + (cc + 1) * 128, :])
                for dc in range(4):
                    nc.sync.dma_start(
                        woT_sb[dc][:], woT_d.ap()[dc * 128:(dc + 1) * 128, :])
            else:
                for cc in range(NCH):
                    nc.sync.dma_start(
                        xT[cc][:],
                        xT_d.ap()[b * C + cc * 128:b * C + (cc + 1) * 128, :])

            q_all = qkv_pool.tile([128, 6 * D], bf16, tag="q")
            k_all = qkv_pool.tile([128, 6 * D], bf16, tag="k")
            v_all = qkv_pool.tile([128, 6 * D], bf16, tag="v")
            qkv_all = (q_all, k_all, v_all)
            # proj-phases k, v, q; per (proj, p) one accumulation chain whose
            # PSUM->SBUF copy overlaps the next chain's matmuls
            for j in (1, 2, 0):
                for p in range(P):
                    psj = qkv_psum.tile([128, D], f32, name="ps",
                                        tag="qkvps", bufs=3)
                    if j == 0:
                        # q bias via K=1 ones-row matmul
                        nc.tensor.matmul(psj[:], lhsT=ones_sb[:, 0:128],
                                         rhs=bq_sb[:], start=True, stop=False)
                    for cc in range(NCH):
                        nc.tensor.matmul(
                            psj[:], lhsT=xT[cc][:, p * 128:(p + 1) * 128],
                            rhs=wT_sb[cc][:, j * D:(j + 1) * D],
                            start=(cc == 0 and j > 0), stop=(cc == NCH - 1))
                    nc.scalar.activation(
                        qkv_all[j][:, p * D:(p + 1) * D], psj[:], COPY)
            return q_all, k_all, v_all

        def emit_attn(b, q_all, k_all, v_all):
            """Attention stage, software-pipelined over p: qk(p) runs while
            softmax(p-1) and att*v(p-2) drain, keeping the DVE dense despite
            the ScalarE hops (exp, broadcast-expand)."""
            qk = sm_pool.tile([128, P * G * P], f32, tag="qk")   # [128, 288]
            attn = sm_pool.tile([128, 288], bf16, tag="attn")
            ssum = sm_pool.tile([128, 48], f32, tag="ssum")
            od = []

            def qk_ops(p):
                for h in range(2):  # q' half
                    tmp = tmp_pool.tile([128, 3 * D], bf16, tag="tmp")
                    in0 = ap(q_all[:], p * D,
                             [[6 * D, 128], [0, 3], [FD, G], [1, FD]])
                    in1 = ap(k_all[:], h * 3 * D,
                             [[6 * D, 128], [D, 3], [FD, G], [1, FD]])
                    o3 = ap(tmp[:], 0, [[3 * D, 128], [D, 3], [FD, G], [1, FD]])
                    nc.vector.tensor_tensor(o3, in0, in1, op=MULT)
                    # log2 add-tree over f (bf16 2x-mode TT beats 1x reduce)
                    w = FD
                    while w > 2:
                        w //= 2
                        nc.vector.tensor_tensor(
                            ap(tmp[:], 0, [[3 * D, 128], [FD, 24], [1, w]]),
                            ap(tmp[:], 0, [[3 * D, 128], [FD, 24], [1, w]]),
                            ap(tmp[:], w, [[3 * D, 128], [FD, 24], [1, w]]),
                            op=ADD)
                    nc.vector.tensor_tensor(
                        ap(qk[:], p * 48 + h * 3, [[288, 128], [1, 3], [6, G]]),
                        ap(tmp[:], 0, [[3 * D, 128], [D, 3], [FD, G]]),
                        ap(tmp[:], 1, [[3 * D, 128], [D, 3], [FD, G]]),
                        op=ADD)

            def sm_ops(p):
                # per-p softmax over q'; no max-subtraction (logits bounded)
                nc.scalar.activation(qk[:, p * 48:(p + 1) * 48],
                                     qk[:, p * 48:(p + 1) * 48], EXP,
                                     scale=0.125)
                nc.vector.tensor_reduce(
                    ssum[:, p * G:(p + 1) * G],
                    ap(qk[:], p * 48, [[288, 128], [6, G], [1, 6]]),
                    axis=AX, op=ADD)
                nc.vector.reciprocal(ssum[:, p * G:(p + 1) * G],
                                     ssum[:, p * G:(p + 1) * G])
                nc.vector.tensor_tensor(
                    ap(attn[:], p * 48, [[288, 128], [6, G], [1, 6]]),
                    ap(qk[:], p * 48, [[288, 128], [6, G], [1, 6]]),
                    ap(ssum[:], p * G, [[48, 128], [1, G], [0, 6]]), op=MULT)

            def av_ops(p):
                # out^T[n,(g,f)] = sum_q' attn[n,(p,g,q')] * v[n,(q',g,f)]
                outT = outT_pool.tile([128, D], bf16, name="outT", tag="outT")
                for h in range(2):  # g half
                    tmp2 = tmp_pool.tile([128, 3 * D], bf16, tag="tmp")
                    a0 = ap(attn[:], p * 48 + h * 4 * P,
                            [[288, 128], [1, 6], [6, 4], [0, FD]])
                    v0 = ap(v_all[:], h * 4 * FD,
                            [[6 * D, 128], [D, 6], [FD, 4], [1, FD]])
                    t0 = ap(tmp2[:], 0, [[3 * D, 128], [256, 6], [FD, 4],
                                         [1, FD]])
                    # broadcast-expand attn over f on ScalarE (otherwise the
                    # step-0 input AP forces the DVE multiply into 1x mode)
                    nc.scalar.activation(t0, a0, COPY)
                    nc.vector.tensor_tensor(tmp2[:], tmp2[:], v0, op=MULT)
                    # add-tree over q' (6 planes of 256)
                    nc.vector.tensor_tensor(
                        tmp2[:, 0:768], tmp2[:, 0:768], tmp2[:, 768:1536],
                        op=ADD)
                    nc.vector.tensor_tensor(
                        tmp2[:, 0:256], tmp2[:, 0:256], tmp2[:, 512:768],
                        op=ADD)
                    nc.vector.tensor_tensor(
                        ap(outT[:], h * 4 * FD, [[D, 128], [1, 256]]),
                        tmp2[:, 0:256], tmp2[:, 256:512], op=ADD)

                ps = ot_psum.tile([128, D], bf16, name="ps", tag="tps", bufs=2)
                for dc in range(4):
                    nc.tensor.transpose(
                        ps[:, dc * 128:(dc + 1) * 128],
                        outT[:, dc * 128:(dc + 1) * 128],
                        identb[:])
                od_p = od_pool.tile([128, D], bf16, name="od", tag="od")
                nc.scalar.activation(od_p[:], ps[:], COPY)
                od.append(od_p)

            for pp in range(P + 2):
                if pp < P:
                    qk_ops(pp)
                if 1 <= pp <= P:
                    sm_ops(pp - 1)
                if pp >= 2:
                    av_ops(pp - 2)
            return od

        def emit_out(b, od):
            """bf16 output projection + residual into freshly-read xb tiles
            (quarter-major: each store leaves once its residuals land)."""
            r0 = b * NB
            for co in range(4):
                xb = xb_pool.tile([128, QF], bf16, name="xb", tag="xb")
                nc.sync.dma_start(
                    xb[:], xb_d.ap()[r0:r0 + NB, co * QF:(co + 1) * QF])
                for p in range(P):
                    pso = o_psum.tile([128, D], f32, name="pso", tag="ops")
                    for dc in range(4):
                        nc.tensor.matmul(
                            pso[:], lhsT=od[p][:, dc * 128:(dc + 1) * 128],
                            rhs=woT_sb[dc][:, co * D:(co + 1) * D],
                            start=(dc == 0), stop=(dc == 3))
                    xsl = ap(xb[:], p, [[QF, 128], [P, D]])
                    # residual add on DVE (GPSIMD cannot read PSUM); lands in
                    # the out window where the DVE is otherwise idle
                    nc.vector.tensor_tensor(xsl, pso[:], xsl, op=ADD)
                nc.sync.dma_start(
                    out_d.ap()[r0:r0 + NB, co * QF:(co + 1) * QF], xb[:])

        if reps == 0:
            # timing-baseline null program: same I/O tensors, trivial work
            z = xT_pool.tile([128, TOK], bf16, name="xT", tag="xT")
            nc.sync.dma_start(z[:, 0:64], AP(xT_d, 0, [[TOK, 128], [1, 64]]))
            zf = xb_pool.tile([128, QF], bf16, name="xb", tag="xb")
            nc.gpsimd.memset(zf[:, 0:64], 0)
            nc.sync.dma_start(AP(out_d, 0, [[CP, 128], [1, 64]]), zf[:, 0:64])
            load_weights()
            nb_total = 0
        else:
            nb_total = nblocks * reps

        # 2-stage software pipeline: head (PE projections) and
        # attention+output trailing by one block.
        hcarry = None
        for i in range(nb_total + 1):
            nxt_h = None
            if i < nb_total:
                nxt_h = (i % nblocks, emit_head(i % nblocks))
            if hcarry is not None:
                hb, h = hcarry
                od = emit_attn(hb, *h)
                emit_out(hb, od)
            hcarry = nxt_h

    _dedupe_ldweights(nc, mybir)
    nc.compile()
    return nc


def _dedupe_ldweights(nc, mybir):
    """Drop InstLdweights whose weights AP is identical to the previous one
    on the PE stream (no intervening transpose, which reloads the array).
    Waits/updates on a dropped LDW are merged into the following matmul's
    sync_info (multi-wait is legal pre-compile; generate_event_semaphores
    splits them later)."""

    def apkey(a):
        return (str(a.memref), str(a.offset), str(a.ap), str(a.dtype))

    for blk in nc.m.functions[0].blocks:
        insts = blk.instructions
        last = None
        drop = set()
        pending_sync = []
        for idx, ins in enumerate(insts):
            nm = type(ins).__name__
            if nm == "InstLdweights":
                key = (apkey(ins.ins[0]), str(ins.perf_mode),
                       str(ins.is_transpose), str(ins.tile_position))
                if key == last:
                    drop.add(idx)
                    if ins.sync_info is not None:
                        pending_sync.append(ins.sync_info)
                last = key
            elif nm == "InstMatmult":
                if getattr(ins, "is_transpose", False):
                    last = None
                if pending_sync:
                    si = ins.sync_info
                    if si is None:
                        si = mybir.SyncInfo(on_wait=[], on_update=[])
                    for extra in pending_sync:
                        si.on_wait = list(si.on_wait) + list(extra.on_wait)
                        si.on_update = (list(si.on_update)
                                        + list(extra.on_update))
                    ins.sync_info = si
                    pending_sync = []
        if drop:
            assert not pending_sync
            keep = [i for idx, i in enumerate(insts) if idx not in drop]
            del insts[:]
            insts.extend(keep)


def get_program(ns, reps=1):
    key = (ns, reps)
    if key not in _CACHE:
        _CACHE[key] = _build(ns, reps)
    return _CACHE[key]


def _host_prep(inputs):
    """Host-side weight/bias prep (shared across cores)."""
    bf = ml_dtypes.bfloat16
    wq = np.asarray(inputs["wq"], np.float32)
    wk = np.asarray(inputs["wk"], np.float32)
    wv = np.asarray(inputs["wv"], np.float32)
    wo = np.asarray(inputs["wo"], np.float32)
    wT = np.ascontiguousarray(
        np.concatenate([wq.T, wk.T, wv.T], axis=1)).astype(bf)      # [C, 3D]
    woT = np.ascontiguousarray(wo.T).astype(bf)                      # [D, C]
    # k-bias is softmax-invariant (adds a row-constant to the logits);
    # v-bias passes through attention unchanged (sum(attn)==1) so it folds
    # into the output-projection bias: bo_eff = bo + wo @ bv.
    bq = np.asarray(inputs["bq"], np.float32).reshape(1, D).astype(bf)
    bo_eff = (np.asarray(inputs["bo"], np.float32)
              + np.asarray(wo, np.float64) @ np.asarray(inputs["bv"],
                                                        np.float64)
              ).astype(np.float32)
    return wT, woT, bq, bo_eff


def _host_x_prep(xs, bo_eff):
    """Per-core x prep: c-major block-major bf16 plane + n-major residual
    stream with the output bias folded in."""
    bf = ml_dtypes.bfloat16
    ns = xs.shape[0]
    nb = ns // NB
    # [ns, C, P] -> [nb, C, P, NB] -> [nb*C, TOK]
    xT = np.ascontiguousarray(
        xs.reshape(nb, NB, C, P).transpose(0, 2, 3, 1)).reshape(
            nb * C, TOK).astype(bf)
    xb = (xs + bo_eff[None, :, None]).astype(bf).reshape(ns, CP)
    return xT, np.ascontiguousarray(xb)


def kernel(**inputs):
    from concourse.bass_utils import run_bass_kernel_spmd

    x = np.asarray(inputs["parts_feat"], np.float32)
    n_total = x.shape[0]
    xs_all = x.reshape(n_total, C, P)
    ns = n_total // NCORES
    wT, woT, bq, bo_eff = _host_prep(inputs)

    nc = get_program(ns)
    in_maps = []
    for i in range(NCORES):
        xT, xb = _host_x_prep(xs_all[i * ns:(i + 1) * ns], bo_eff)
        in_maps.append({
            "xT": xT, "xb": xb, "wT": wT, "woT": woT, "bq": bq,
        })
    res = run_bass_kernel_spmd(nc, in_maps, core_ids=list(range(NCORES)))
    out = np.concatenate([r["out"] for r in res.results], axis=0)
    # reference() squeezes the trailing singleton: output is [N, C, P]
    return out.astype(np.float32).reshape(n_total, C, P)
